# revision 1
# baseline (speedup 1.0000x reference)
"""CapsuleTransformConv on 8 Trainium2 NeuronCores.

Problem:  x [4,16,16,32,16] f32, matrix [288,16,512] f32.
          im2col (K=3, VALID) -> tile [4,14,14,288,16]
          votes  = einsum('bhwna,nac->bhwnc', tile, matrix)
          out    = votes.reshape(4,14,14,288,32,16)

Sharding: tensor-parallel over the filter*atom output axis (512 -> 64 per
core).  Every core reads the full x (2 MB) and its 64-wide slice of the
weights; writes its [784, 288, 64] slice of the output (~58 MB, the
dominant HBM traffic).

Per-core kernel (~253 us HW, vs ~208 us pure write time at the measured
~290 GB/s per-core effective HBM write rate with all 8 cores active):
  - x is loaded once (2 DMAs) and PE-transposed into 4 per-octet tiles
    xT[(c_in_octet, atom)=128 partitions, (b,h,w)=1024]; x is read from
    HBM exactly once.
  - Per tap (ki,kj), GPSIMD compacts the im2col gather into
    tap[(dc,a), oct*784 + (b,i,j)] so every matmul's stationary operand
    is a flat contiguous slice (walrus requires a single free dim).
  - Weights for 8 consecutive capsules (one c-octet of one tap) are laid
    out block-diagonally in a [128, 512] f32r tile so one K=128 matmul
    computes 8 independent [pos,16]@[16,64] capsule matmuls.  FP32r
    matmul inputs must be produced by a rounding instruction (never by
    DMA), so paint DMAs land in a reused memset-once f32 buffer and a
    full-partition DVE copy rounds each 4-group chunk into its per-tap
    wpack tile.
  - Main loop: 9 taps x (4 batches x 2 i-windows); each iteration runs
    4 matmuls (c-octets) into one 4-bank PSUM tile, a PSUM->SBUF copy
    split by bank pairs across Vector||Scalar, and one contiguous
    0.7-0.9 MB DMA to the tap-major output, alternating the two HWDGE
    rings.
  - Matmuls run in float32r (TF32-class, 1 cyc/row vs 4 for fp32);
    fp32 accumulation in PSUM; rel err vs fp32 reference ~1.7e-4.
    Set MM_MODE="f32" for bit-exact output at ~303 us.
"""

import numpy as np

B, H, W, C, A = 4, 16, 16, 32, 16
KS = 3
OH = OW = 14
NCAP = KS * KS * C          # 288 capsules
FTOT = 512                  # filter*atom
NCORES = 8
FPC = FTOT // NCORES        # 64 output features per core
POS = B * OH * OW           # 784 output positions
NG = NCAP // 8              # 36 groups of 8 capsules = (tap, c-octet)

_NC_CACHE = {}
MM_MODE = "f32r"  # "f32" (exact, 4 cyc/row) or "f32r" (TF32-class, 1 cyc/row)


def _build_nc(mm_f32r=True):
    import concourse.bass as bass  # noqa: F401
    import concourse.mybir as mybir
    import concourse.tile as tile
    from concourse import bacc, masks

    f32 = mybir.dt.float32
    mmdt = mybir.dt.float32r if mm_f32r else mybir.dt.float32

    nc = bacc.Bacc(None, target_bir_lowering=False)
    x_d = nc.declare_dram_parameter("x", [B, H, W, C, A], f32, isOutput=False)
    m_d = nc.declare_dram_parameter("mat", [NCAP, A, FPC], f32, isOutput=False)
    # Tap-major output layout: out[kk, pos, 32*64].  Each inner-loop DMA then
    # writes one fully contiguous ~0.7-0.9 MB block (vs 8 KB runs strided by
    # 72 KB in pos-major layout); the host transposes kk back into n.
    o_d = nc.declare_dram_parameter("out", [KS * KS, POS, 32 * FPC], f32,
                                    isOutput=True)

    x2d = x_d.rearrange("b h w c a -> (b h w) (c a)")   # [1024, 512]

    with tile.TileContext(nc) as tc:
        with (
            tc.tile_pool(name="const", bufs=1) as constp,
            tc.tile_pool(name="big", bufs=1) as bigp,
            tc.tile_pool(name="stage", bufs=3) as stagep,
            tc.tile_pool(name="tapp", bufs=2) as tapp,
            tc.tile_pool(name="psum", bufs=2, space="PSUM") as psump,
        ):
            ident = constp.tile([128, 128], f32, tag="ident")
            masks.make_identity(nc, ident[:])

            # ---- weights chunk 0 paint: first in the sync ring ----
            # (moved ahead of the x loads; see the wpack build below)
            msrc = m_d.rearrange("(g gc) a f -> gc a g f", gc=8)
            wtmp = bigp.tile([128, 16 * 512], f32, tag="wtmp")
            # Small memset on DVE (idle, early) so round-0 paints go first.
            nc.vector.memset(wtmp[:, 0:2048], 0.0)
            nc.gpsimd.memset(wtmp[:, 2048:], 0.0)
            wtv = wtmp[:].rearrange("p (g v) -> p g v", g=16)
            for gc in range(8):
                nc.sync.dma_start(
                    wtv[gc * 16:(gc + 1) * 16, 0:4, gc * FPC:(gc + 1) * FPC],
                    msrc[gc, :, 0:4, :],
                )

            # ---- x: HBM -> SBUF once, four 2-slab tiles [128, 2, 512] ----
            # (per-batch granularity: batch b's transposes depend only on
            # tile b, so the first matmul chain starts ~2us after the first
            # 512 KB lands)
            xsrc = x2d.rearrange("(t s p) c -> t p s c", t=4, p=128)
            x_sbs = [
                bigp.tile([128, 2 * 512], f32, tag=f"x_sb{t}", name=f"x_sb{t}")
                for t in range(4)
            ]
            for t in range(4):
                nc.sync.dma_start(
                    x_sbs[t][:].rearrange("p (s c) -> p s c", s=2), xsrc[t]
                )

            # ---- weights: block-diagonal wpack, built per-tap ----
            # wpack_c[(gc,a), oct*512 + gc*64 + f] = matrix[(c*4+oct)*8+gc, a, f]
            # else 0.  FP32r matmul inputs must be produced by a rounding
            # instruction (never by DMA), so paint DMAs land in transient f32
            # tiles and a full-partition engine copy rounds each chunk.
            # One chunk per tap kk so kk=0 matmuls start without waiting for
            # the whole weight build.  The two transient tiles are memset
            # once: every chunk paints the same diagonal positions, so the
            # off-diagonal zeros stay clean across reuse.
            # One serially-reused paint buffer covering 4 taps (16 groups);
            # every round paints the same diagonal positions, so the memset
            # zeros stay clean across reuse.  Round 0 (tap 0) was painted
            # above, ahead of the x loads.
            wpacks = []
            for rnd, ntap in ((0, 1), (1, 4), (2, 4)):
                g0 = (0, 4, 20)[rnd]  # first group of this round
                ng = ntap * 4
                if rnd > 0:
                    for gc in range(8):
                        # Scalar ring: idle until outputs begin.
                        nc.scalar.dma_start(
                            wtv[gc * 16:(gc + 1) * 16, 0:ng,
                                gc * FPC:(gc + 1) * FPC],
                            msrc[gc, :, g0: g0 + ng, :],
                        )
                for t in range(ntap):
                    kk_of = g0 // 4 + t
                    wp = bigp.tile(
                        [128, 4 * 512], mmdt,
                        tag=f"wpack{kk_of}", name=f"wpack{kk_of}",
                    )
                    nc.vector.tensor_copy(
                        wp[:], wtmp[:, t * 2048:(t + 1) * 2048]
                    )
                    wpacks.append(wp)

            # ---- xT: PE-transpose x into 4 per-octet tiles [(dc,a), (b,h,w)]
            # Separate tiles so each octet's im2col cast can start as soon as
            # its own 8 transposes land.
            xts = [
                bigp.tile([128, 1024], f32, tag=f"xt{o}", name=f"xt{o}")
                for o in range(4)
            ]
            for s in range(8):
                for oct in range(4):
                    tr = psump.tile([128, 128], f32, tag="mm")
                    nc.tensor.transpose(
                        tr[:],
                        x_sbs[s // 2][
                            :, (s % 2) * 512 + oct * 128:
                            (s % 2) * 512 + (oct + 1) * 128
                        ],
                        ident[:],
                    )
                    dst = xts[oct][:, s * 128:(s + 1) * 128]
                    if (s + oct) % 2 == 0:
                        nc.vector.tensor_copy(dst, tr[:])
                    else:
                        nc.scalar.copy(dst, tr[:])

            xtvs = [
                t[:].rearrange("p (b h w) -> p b h w", b=B, h=H) for t in xts
            ]

            # ---- main loop: 9 taps (outer) x per-batch pos windows ----
            # The matmul stationary operand must be a single flat free dim
            # (walrus constraint), so per tap we compact the im2col gather
            # into tap[(dc,a), oct*784 + (b,i,j)] with GPSIMD copies.
            it = 0
            for kk in range(9):
                ki, kj = kk // 3, kk % 3
                tap = tapp.tile([128, 4 * POS], mmdt, tag="tap")
                for oct in range(4):
                    dst = tap[:, oct * POS:(oct + 1) * POS].rearrange(
                        "p (b i j) -> p b i j", b=B, i=OH
                    )
                    src = xtvs[oct][:, :, ki: ki + OH, kj: kj + OW]
                    if kk == 0:
                        # First tap per-batch on DVE/ACT (idle at startup):
                        # batch b's cast only needs x slabs 2b..2b+1, so the
                        # first matmul starts as soon as the first slabs
                        # transpose.  Later taps prefetch on idle GPSIMD.
                        for bb in range(B):
                            if (oct + bb) % 2 == 0:
                                nc.vector.tensor_copy(
                                    dst[:, bb], src[:, bb]
                                )
                            else:
                                nc.scalar.copy(dst[:, bb], src[:, bb])
                    else:
                        nc.gpsimd.tensor_copy(dst, src)
                for b in range(B):
                    for i0, ni in ((0, 8), (8, 6)):
                        m = ni * OW  # 112 or 84 output positions
                        ps = psump.tile([128, 2048], f32, tag="mm")
                        for oct in range(4):
                            off = oct * POS + b * (OH * OW) + i0 * OW
                            nc.tensor.matmul(
                                ps[0:m, oct * 512:(oct + 1) * 512],
                                tap[:, off: off + m],
                                wpacks[kk][:, oct * 512:(oct + 1) * 512],
                                start=True,
                                stop=True,
                            )
                        st = stagep.tile([128, 2048], f32, tag="st")
                        # Split the PSUM->SBUF copy by bank pairs so DVE and
                        # ACT run in parallel (different PSUM banks).
                        nc.vector.tensor_copy(st[0:m, 0:1024], ps[0:m, 0:1024])
                        nc.scalar.copy(st[0:m, 1024:2048], ps[0:m, 1024:2048])
                        # Alternate the two HWDGE rings (SP / ACT) so output
                        # DMAs pipeline across both.
                        dma_eng = nc.sync if it % 2 == 0 else nc.scalar
                        q0 = b * (OH * OW) + i0 * OW
                        dma_eng.dma_start(
                            o_d[kk, q0: q0 + m, :],
                            st[0:m, :],
                        )
                        it += 1

    nc.compile()
    return nc


def _get_nc():
    key = MM_MODE
    if key not in _NC_CACHE:
        _NC_CACHE[key] = _build_nc(mm_f32r=(MM_MODE == "f32r"))
    return _NC_CACHE[key]


def kernel(x, matrix):
    from concourse.bass_utils import run_bass_kernel_spmd

    x = np.ascontiguousarray(x, dtype=np.float32)
    matrix = np.ascontiguousarray(matrix, dtype=np.float32)
    nc = _get_nc()
    in_maps = [
        {
            "x": x,
            "mat": np.ascontiguousarray(matrix[:, :, c * FPC:(c + 1) * FPC]),
        }
        for c in range(NCORES)
    ]
    r = run_bass_kernel_spmd(nc, in_maps, list(range(NCORES)))
    # parts[c]: [9, 784, 2048] tap-major -> [784, kk, 32, core, 64] -> full
    arr = np.stack([r.results[c]["out"] for c in range(NCORES)])
    arr = arr.reshape(NCORES, KS * KS, POS, 32, FPC)
    arr = arr.transpose(2, 1, 3, 0, 4)               # [pos, kk, 32, core, f]
    full = arr.reshape(POS, NCAP, FTOT)
    return np.ascontiguousarray(
        full.reshape(B, OH, OW, NCAP, 32, 16).astype(np.float32)
    )



# revision 5
# speedup vs baseline: 1.2112x; 1.2112x over previous
"""CapsuleTransformConv on 8 Trainium2 NeuronCores.

Problem:  x [4,16,16,32,16] f32, matrix [288,16,512] f32.
          im2col (K=3, VALID) -> tile [4,14,14,288,16]
          votes  = einsum('bhwna,nac->bhwnc', tile, matrix)
          out    = votes.reshape(4,14,14,288,32,16)

Sharding: tensor-parallel over the filter*atom output axis (512 -> 64 per
core).  Every core reads the full x (2 MB) and its 64-wide slice of the
weights; writes its [784, 288, 64] slice of the output (~58 MB, the
dominant HBM traffic).

Per-core kernel (~253 us HW, vs ~208 us pure write time at the measured
~290 GB/s per-core effective HBM write rate with all 8 cores active):
  - x is loaded once (2 DMAs) and PE-transposed into 4 per-octet tiles
    xT[(c_in_octet, atom)=128 partitions, (b,h,w)=1024]; x is read from
    HBM exactly once.
  - Per tap (ki,kj), GPSIMD compacts the im2col gather into
    tap[(dc,a), oct*784 + (b,i,j)] so every matmul's stationary operand
    is a flat contiguous slice (walrus requires a single free dim).
  - Weights for 8 consecutive capsules (one c-octet of one tap) are laid
    out block-diagonally in a [128, 512] f32r tile so one K=128 matmul
    computes 8 independent [pos,16]@[16,64] capsule matmuls.  FP32r
    matmul inputs must be produced by a rounding instruction (never by
    DMA), so paint DMAs land in a reused memset-once f32 buffer and a
    full-partition DVE copy rounds each 4-group chunk into its per-tap
    wpack tile.
  - Main loop: 9 taps x (4 batches x 2 i-windows); each iteration runs
    4 matmuls (c-octets) into one 4-bank PSUM tile, a PSUM->SBUF copy
    split by bank pairs across Vector||Scalar, and one contiguous
    0.7-0.9 MB DMA to the tap-major output, alternating the two HWDGE
    rings.
  - Matmuls run in float32r (TF32-class, 1 cyc/row vs 4 for fp32);
    fp32 accumulation in PSUM; rel err vs fp32 reference ~1.7e-4.
    Set MM_MODE="f32" for bit-exact output at ~303 us.
"""

import numpy as np

B, H, W, C, A = 4, 16, 16, 32, 16
KS = 3
OH = OW = 14
NCAP = KS * KS * C          # 288 capsules
FTOT = 512                  # filter*atom
NCORES = 8
FPC = FTOT // NCORES        # 64 output features per core
POS = B * OH * OW           # 784 output positions
NG = NCAP // 8              # 36 groups of 8 capsules = (tap, c-octet)

_NC_CACHE = {}
MM_MODE = "f32r"  # "f32" (exact, 4 cyc/row) or "f32r" (TF32-class, 1 cyc/row)


def _build_nc(mm_f32r=True):
    import concourse.bass as bass  # noqa: F401
    import concourse.mybir as mybir
    import concourse.tile as tile
    from concourse import bacc, masks

    f32 = mybir.dt.float32
    bf16 = mybir.dt.bfloat16
    mmdt = mybir.dt.float32r if mm_f32r else mybir.dt.float32

    nc = bacc.Bacc(None, target_bir_lowering=False)
    x_d = nc.declare_dram_parameter("x", [B, H, W, C, A], f32, isOutput=False)
    m_d = nc.declare_dram_parameter("mat", [NCAP, A, FPC], f32, isOutput=False)
    # Tap-major output layout: out[kk, pos, 32*64].  Each inner-loop DMA then
    # writes one fully contiguous ~0.7-0.9 MB block (vs 8 KB runs strided by
    # 72 KB in pos-major layout); the host transposes kk back into n.
    # bf16 output: halves the dominant HBM write traffic (~58 MB -> ~29 MB
    # per core); the host upcasts back to f32.  Worst-case bf16 rounding is
    # 2^-8 relative, far inside the 2e-2 gate.
    o_d = nc.declare_dram_parameter("out", [KS * KS, POS, 32 * FPC], bf16,
                                    isOutput=True)

    x2d = x_d.rearrange("b h w c a -> (b h w) (c a)")   # [1024, 512]

    with tile.TileContext(nc) as tc:
        with (
            tc.tile_pool(name="const", bufs=1) as constp,
            tc.tile_pool(name="big", bufs=1) as bigp,
            tc.tile_pool(name="stage", bufs=3) as stagep,
            tc.tile_pool(name="tapp", bufs=2) as tapp,
            tc.tile_pool(name="psum", bufs=2, space="PSUM") as psump,
        ):
            ident = constp.tile([128, 128], f32, tag="ident")
            masks.make_identity(nc, ident[:])

            # ---- weights chunk 0 paint: first in the sync ring ----
            # (moved ahead of the x loads; see the wpack build below)
            msrc = m_d.rearrange("(g gc) a f -> gc a g f", gc=8)
            wtmp = bigp.tile([128, 16 * 512], f32, tag="wtmp")
            # Small memset on DVE (idle, early) so round-0 paints go first.
            nc.vector.memset(wtmp[:, 0:2048], 0.0)
            nc.gpsimd.memset(wtmp[:, 2048:], 0.0)
            wtv = wtmp[:].rearrange("p (g v) -> p g v", g=16)
            for gc in range(8):
                nc.sync.dma_start(
                    wtv[gc * 16:(gc + 1) * 16, 0:4, gc * FPC:(gc + 1) * FPC],
                    msrc[gc, :, 0:4, :],
                )

            # ---- x: HBM -> SBUF once, four 2-slab tiles [128, 2, 512] ----
            # (per-batch granularity: batch b's transposes depend only on
            # tile b, so the first matmul chain starts ~2us after the first
            # 512 KB lands)
            xsrc = x2d.rearrange("(t s p) c -> t p s c", t=4, p=128)
            x_sbs = [
                bigp.tile([128, 2 * 512], f32, tag=f"x_sb{t}", name=f"x_sb{t}")
                for t in range(4)
            ]
            for t in range(4):
                nc.sync.dma_start(
                    x_sbs[t][:].rearrange("p (s c) -> p s c", s=2), xsrc[t]
                )

            # ---- weights: block-diagonal wpack, built per-tap ----
            # wpack_c[(gc,a), oct*512 + gc*64 + f] = matrix[(c*4+oct)*8+gc, a, f]
            # else 0.  FP32r matmul inputs must be produced by a rounding
            # instruction (never by DMA), so paint DMAs land in transient f32
            # tiles and a full-partition engine copy rounds each chunk.
            # One chunk per tap kk so kk=0 matmuls start without waiting for
            # the whole weight build.  The two transient tiles are memset
            # once: every chunk paints the same diagonal positions, so the
            # off-diagonal zeros stay clean across reuse.
            # One serially-reused paint buffer covering 4 taps (16 groups);
            # every round paints the same diagonal positions, so the memset
            # zeros stay clean across reuse.  Round 0 (tap 0) was painted
            # above, ahead of the x loads.
            wpacks = []
            for rnd, ntap in ((0, 1), (1, 4), (2, 4)):
                g0 = (0, 4, 20)[rnd]  # first group of this round
                ng = ntap * 4
                if rnd > 0:
                    for gc in range(8):
                        # Scalar ring: idle until outputs begin.
                        nc.scalar.dma_start(
                            wtv[gc * 16:(gc + 1) * 16, 0:ng,
                                gc * FPC:(gc + 1) * FPC],
                            msrc[gc, :, g0: g0 + ng, :],
                        )
                for t in range(ntap):
                    kk_of = g0 // 4 + t
                    wp = bigp.tile(
                        [128, 4 * 512], mmdt,
                        tag=f"wpack{kk_of}", name=f"wpack{kk_of}",
                    )
                    nc.vector.tensor_copy(
                        wp[:], wtmp[:, t * 2048:(t + 1) * 2048]
                    )
                    wpacks.append(wp)

            # ---- xT: PE-transpose x into 4 per-octet tiles [(dc,a), (b,h,w)]
            # Separate tiles so each octet's im2col cast can start as soon as
            # its own 8 transposes land.
            xts = [
                bigp.tile([128, 1024], f32, tag=f"xt{o}", name=f"xt{o}")
                for o in range(4)
            ]
            for s in range(8):
                for oct in range(4):
                    tr = psump.tile([128, 128], f32, tag="mm")
                    nc.tensor.transpose(
                        tr[:],
                        x_sbs[s // 2][
                            :, (s % 2) * 512 + oct * 128:
                            (s % 2) * 512 + (oct + 1) * 128
                        ],
                        ident[:],
                    )
                    dst = xts[oct][:, s * 128:(s + 1) * 128]
                    if (s + oct) % 2 == 0:
                        nc.vector.tensor_copy(dst, tr[:])
                    else:
                        nc.scalar.copy(dst, tr[:])

            xtvs = [
                t[:].rearrange("p (b h w) -> p b h w", b=B, h=H) for t in xts
            ]

            # ---- main loop: 9 taps (outer) x per-batch pos windows ----
            # The matmul stationary operand must be a single flat free dim
            # (walrus constraint), so per tap we compact the im2col gather
            # into tap[(dc,a), oct*784 + (b,i,j)] with GPSIMD copies.
            it = 0
            for kk in range(9):
                ki, kj = kk // 3, kk % 3
                tap = tapp.tile([128, 4 * POS], mmdt, tag="tap")
                for oct in range(4):
                    dst = tap[:, oct * POS:(oct + 1) * POS].rearrange(
                        "p (b i j) -> p b i j", b=B, i=OH
                    )
                    src = xtvs[oct][:, :, ki: ki + OH, kj: kj + OW]
                    if kk == 0:
                        # First tap per-batch on DVE/ACT (idle at startup):
                        # batch b's cast only needs x slabs 2b..2b+1, so the
                        # first matmul starts as soon as the first slabs
                        # transpose.  Later taps prefetch on idle GPSIMD.
                        for bb in range(B):
                            if (oct + bb) % 2 == 0:
                                nc.vector.tensor_copy(
                                    dst[:, bb], src[:, bb]
                                )
                            else:
                                nc.scalar.copy(dst[:, bb], src[:, bb])
                    else:
                        nc.gpsimd.tensor_copy(dst, src)
                for b in range(B):
                    for i0, ni in ((0, 8), (8, 6)):
                        m = ni * OW  # 112 or 84 output positions
                        ps = psump.tile([128, 2048], f32, tag="mm")
                        for oct in range(4):
                            off = oct * POS + b * (OH * OW) + i0 * OW
                            nc.tensor.matmul(
                                ps[0:m, oct * 512:(oct + 1) * 512],
                                tap[:, off: off + m],
                                wpacks[kk][:, oct * 512:(oct + 1) * 512],
                                start=True,
                                stop=True,
                            )
                        st = stagep.tile([128, 2048], bf16, tag="st")
                        # Split the PSUM->SBUF copy by bank pairs so DVE and
                        # ACT run in parallel (different PSUM banks).
                        nc.vector.tensor_copy(st[0:m, 0:1024], ps[0:m, 0:1024])
                        nc.scalar.copy(st[0:m, 1024:2048], ps[0:m, 1024:2048])
                        # Alternate the two HWDGE rings (SP / ACT) so output
                        # DMAs pipeline across both.
                        dma_eng = nc.sync if it % 2 == 0 else nc.scalar
                        q0 = b * (OH * OW) + i0 * OW
                        dma_eng.dma_start(
                            o_d[kk, q0: q0 + m, :],
                            st[0:m, :],
                        )
                        it += 1

    nc.compile()
    return nc


def _get_nc():
    key = MM_MODE
    if key not in _NC_CACHE:
        _NC_CACHE[key] = _build_nc(mm_f32r=(MM_MODE == "f32r"))
    return _NC_CACHE[key]


def kernel(x, matrix):
    from concourse.bass_utils import run_bass_kernel_spmd

    x = np.ascontiguousarray(x, dtype=np.float32)
    matrix = np.ascontiguousarray(matrix, dtype=np.float32)
    nc = _get_nc()
    in_maps = [
        {
            "x": x,
            "mat": np.ascontiguousarray(matrix[:, :, c * FPC:(c + 1) * FPC]),
        }
        for c in range(NCORES)
    ]
    r = run_bass_kernel_spmd(nc, in_maps, list(range(NCORES)))
    # parts[c]: [9, 784, 2048] tap-major -> [784, kk, 32, core, 64] -> full
    arr = np.stack(
        [np.asarray(r.results[c]["out"]).astype(np.float32) for c in range(NCORES)]
    )
    arr = arr.reshape(NCORES, KS * KS, POS, 32, FPC)
    arr = arr.transpose(2, 1, 3, 0, 4)               # [pos, kk, 32, core, f]
    full = arr.reshape(POS, NCAP, FTOT)
    return np.ascontiguousarray(
        full.reshape(B, OH, OW, NCAP, 32, 16).astype(np.float32)
    )



# revision 6
# speedup vs baseline: 1.2421x; 1.0255x over previous
"""CapsuleTransformConv on 8 Trainium2 NeuronCores.

Problem:  x [4,16,16,32,16] f32, matrix [288,16,512] f32.
          im2col (K=3, VALID) -> tile [4,14,14,288,16]
          votes  = einsum('bhwna,nac->bhwnc', tile, matrix)
          out    = votes.reshape(4,14,14,288,32,16)

Sharding: tensor-parallel over the filter*atom output axis (512 -> 64 per
core).  Every core reads the full x and its 64-wide weight slice; writes
its [784, 288, 64] output slice (~29 MB bf16, the dominant HBM traffic).

Per-core kernel structure (v3):
  - Host marshals inputs: x is pre-transposed to 4 per-octet fp16 tiles
    xt[oct][(dc,a)=128, (b,h,w)=1024] and the weights are pre-packed into
    9 block-diagonal fp16 tiles wp[kk][(gc,a)=128, oct*512+gc*64+f] (one
    K=128 matmul computes 8 independent [pos,16]@[16,64] capsule matmuls).
    This removes the on-device PE transposes, scattered weight-paint DMAs
    and f32r rounding copies entirely.
  - Instead of 9 per-tap im2col compactions, GPSIMD builds only 3
    kj-shifted tensors shift[kj][(dc,a), (oct,b,h16,j14)]; the three ki
    taps of a kj slice them as contiguous row windows (walrus needs a
    flat stationary slice).  3x less GPSIMD gather work, built one kj
    ahead, per-(b,oct) granularity so the first matmul starts ~3us in.
  - Main loop: 3 kj x 3 ki x (4 b x 2 i-windows); 4 matmuls (c-octets)
    into a double-buffered 4-bank PSUM tile, PSUM->SBUF drain split
    DVE[0:960] || ACT[960:2048] converting f32 -> bf16, then one
    contiguous ~0.2-0.3 MB DMA per window to the tap-major bf16 output.
  - All output DMAs issue on the Sync queue only: a dma_start on the ACT
    queue waits on the DVE half-drain and stalls the next ACT drain
    behind it (in-order queue), which was the v2 serializer.
  - fp16 matmul inputs (1 cyc/row), f32 PSUM accumulate, bf16 output
    write; host upcasts to f32.  rel err ~2e-3 vs the f32 reference.
"""

import numpy as np

B, H, W, C, A = 4, 16, 16, 32, 16
KS = 3
OH = OW = 14
NCAP = KS * KS * C          # 288 capsules
FTOT = 512                  # filter*atom
NCORES = 8
FPC = FTOT // NCORES        # 64 output features per core
POS = B * OH * OW           # 784 output positions

_NC_CACHE = {}


def _build_nc():
    import concourse.bass as bass  # noqa: F401
    import concourse.mybir as mybir
    import concourse.tile as tile
    from concourse import bacc

    f32 = mybir.dt.float32
    fp16 = mybir.dt.float16
    bf16 = mybir.dt.bfloat16

    nc = bacc.Bacc(None, target_bir_lowering=False)
    xt_d = nc.declare_dram_parameter("xt", [4, 128, B * H * W], fp16,
                                     isOutput=False)
    wp_d = nc.declare_dram_parameter("wp", [KS * KS, 128, 4 * 512], fp16,
                                     isOutput=False)
    # Tap-major output layout: out[kk, pos, 32*64].  Each inner-loop DMA
    # writes one fully contiguous block; the host transposes kk back into n.
    o_d = nc.declare_dram_parameter("out", [KS * KS, POS, 32 * FPC], bf16,
                                    isOutput=True)

    with tile.TileContext(nc) as tc:
        with (
            tc.tile_pool(name="xtp", bufs=1) as xtp,
            tc.tile_pool(name="wpp", bufs=1) as wpp,
            tc.tile_pool(name="shiftp", bufs=2) as shiftp,
            tc.tile_pool(name="stage", bufs=3) as stagep,
            tc.tile_pool(name="psum", bufs=2, space="PSUM") as psump,
        ):
            # ---- weight packs: first-consumed tap first, on the ACT ring
            # (the sync ring carries xt + all output DMAs) ----
            kk_order = [ki * 3 + kj for kj in range(3) for ki in range(3)]
            wps = [None] * (KS * KS)
            for kk in kk_order:
                wp_t = wpp.tile([128, 4 * 512], fp16, tag=f"wp{kk}",
                                name=f"wp{kk}")
                nc.scalar.dma_start(wp_t[:], wp_d[kk])
                wps[kk] = wp_t

            # ---- x: already transposed+fp16 on host; 4 per-octet tiles ----
            xts = [
                xtp.tile([128, B * H * W], fp16, tag=f"xt{o}", name=f"xt{o}")
                for o in range(4)
            ]
            for o in range(4):
                nc.sync.dma_start(xts[o][:], xt_d[o])
            xtvs = [
                t[:].rearrange("p (b h w) -> p b h w", b=B, h=H) for t in xts
            ]

            # ---- kj-shift builds: shift[(dc,a), (oct,b,h,j)] ----
            # j:14-of-16 compaction only; the three ki taps of this kj read
            # contiguous row windows.  Per-(b,oct) GPSIMD copies so early
            # matmuls start as soon as their slice lands.
            def build_shift(kj):
                sh = shiftp.tile([128, 4 * B * H * OW], fp16, tag="shift",
                                 name=f"shift{kj}")
                shv = sh[:].rearrange("p (o b h j) -> p o b h j", o=4, b=B,
                                      h=H)
                for b in range(B):
                    for o in range(4):
                        nc.gpsimd.tensor_copy(
                            shv[:, o, b], xtvs[o][:, b, :, kj:kj + OW]
                        )
                return sh

            # ---- main loop: kj outer so taps ki=0..2 reuse one shift ----
            sh_cur = build_shift(0)
            for kj in range(3):
                sh_next = build_shift(kj + 1) if kj < 2 else None
                for ki in range(3):
                    kk = ki * 3 + kj
                    for b in range(B):
                        for i0, ni in ((0, 8), (8, 6)):
                            m = ni * OW  # 112 or 84 output positions
                            ps = psump.tile([128, 2048], f32, tag="mm")
                            for o in range(4):
                                off = ((o * B + b) * H + ki + i0) * OW
                                nc.tensor.matmul(
                                    ps[0:m, o * 512:(o + 1) * 512],
                                    sh_cur[:, off: off + m],
                                    wps[kk][:, o * 512:(o + 1) * 512],
                                    start=True,
                                    stop=True,
                                )
                            st = stagep.tile([128, 2048], bf16, tag="st")
                            # f32->bf16 drain split across DVE || ACT
                            # (breakpoint balances the two engine clocks).
                            nc.vector.tensor_copy(st[0:m, 0:960],
                                                  ps[0:m, 0:960])
                            nc.scalar.copy(st[0:m, 960:2048],
                                           ps[0:m, 960:2048])
                            q0 = b * (OH * OW) + i0 * OW
                            nc.sync.dma_start(
                                o_d[kk, q0: q0 + m, :],
                                st[0:m, :],
                            )
                sh_cur = sh_next

    nc.compile()
    return nc


def _get_nc():
    if "v3" not in _NC_CACHE:
        _NC_CACHE["v3"] = _build_nc()
    return _NC_CACHE["v3"]


def make_in_maps(x, matrix):
    """Host-side input marshalling for all 8 cores."""
    x = np.ascontiguousarray(x, dtype=np.float32)
    matrix = np.ascontiguousarray(matrix, dtype=np.float32)
    # xt[oct, (dc,a), (b,h,w)] fp16
    xt = np.ascontiguousarray(
        x.transpose(3, 4, 0, 1, 2).reshape(4, 128, B * H * W)
    ).astype(np.float16)
    in_maps = []
    for c in range(NCORES):
        mr = matrix[:, :, c * FPC:(c + 1) * FPC]          # [288, 16, 64]
        mr = mr.reshape(KS * KS, 4, 8, A, FPC)            # [kk,oct,gc,a,f]
        wp = np.zeros((KS * KS, 128, 4 * 512), np.float16)
        wpv = wp.reshape(KS * KS, 8, A, 4, 8, FPC)        # [kk,gc,a,oct,gc2,f]
        for g in range(8):
            wpv[:, g, :, :, g, :] = mr[:, :, g].transpose(0, 2, 1, 3)
        in_maps.append({"xt": xt, "wp": wp})
    return in_maps


def kernel(x, matrix):
    from concourse.bass_utils import run_bass_kernel_spmd

    nc = _get_nc()
    in_maps = make_in_maps(x, matrix)
    r = run_bass_kernel_spmd(nc, in_maps, list(range(NCORES)))
    # parts[c]: [9, 784, 2048] tap-major -> [pos, kk, 32, core, 64] -> full
    arr = np.stack(
        [np.asarray(r.results[c]["out"]).astype(np.float32) for c in range(NCORES)]
    )
    arr = arr.reshape(NCORES, KS * KS, POS, 32, FPC)
    arr = arr.transpose(2, 1, 3, 0, 4)               # [pos, kk, 32, core, f]
    full = arr.reshape(POS, NCAP, FTOT)
    return np.ascontiguousarray(
        full.reshape(B, OH, OW, NCAP, 32, 16).astype(np.float32)
    )


# revision 8
# speedup vs baseline: 1.5949x; 1.2840x over previous
"""CapsuleTransformConv on 8 Trainium2 NeuronCores.

Problem:  x [4,16,16,32,16] f32, matrix [288,16,512] f32.
          im2col (K=3, VALID) -> tile [4,14,14,288,16]
          votes  = einsum('bhwna,nac->bhwnc', tile, matrix)
          out    = votes.reshape(4,14,14,288,32,16)

Sharding: tensor-parallel over the filter*atom output axis (512 -> 64 per
core).  Every core reads the full x and its 64-wide weight slice; writes
its [784, 288, 64] output slice (~29 MB bf16, the dominant HBM traffic).

Per-core kernel structure (v3):
  - Host marshals inputs: x is pre-transposed to 4 per-octet fp16 tiles
    xt[oct][(dc,a)=128, (b,h,w)=1024] and the weights are pre-packed into
    9 block-diagonal fp16 tiles wp[kk][(gc,a)=128, oct*512+gc*64+f] (one
    K=128 matmul computes 8 independent [pos,16]@[16,64] capsule matmuls).
    This removes the on-device PE transposes, scattered weight-paint DMAs
    and f32r rounding copies entirely.
  - Instead of 9 per-tap im2col compactions, GPSIMD builds only 3
    kj-shifted tensors shift[kj][(dc,a), (oct,b,h16,j14)]; the three ki
    taps of a kj slice them as contiguous row windows (walrus needs a
    flat stationary slice).  3x less GPSIMD gather work, built one kj
    ahead, per-(b,oct) granularity so the first matmul starts ~3us in.
  - Main loop: 3 kj x 3 ki x (4 b x 2 i-windows); 4 matmuls (c-octets)
    into a double-buffered 4-bank PSUM tile, PSUM->SBUF drain split
    DVE[0:960] || ACT[960:2048] converting f32 -> bf16, then one
    contiguous ~0.2-0.3 MB DMA per window to the tap-major bf16 output.
  - All output DMAs issue on the Sync queue only: a dma_start on the ACT
    queue waits on the DVE half-drain and stalls the next ACT drain
    behind it (in-order queue), which was the v2 serializer.
  - fp16 matmul inputs (1 cyc/row), f32 PSUM accumulate, bf16 output
    write; host upcasts to f32.  rel err ~2e-3 vs the f32 reference.
"""

import numpy as np

B, H, W, C, A = 4, 16, 16, 32, 16
KS = 3
OH = OW = 14
NCAP = KS * KS * C          # 288 capsules
FTOT = 512                  # filter*atom
NCORES = 8
FPC = FTOT // NCORES        # 64 output features per core
POS = B * OH * OW           # 784 output positions

_NC_CACHE = {}


def _build_nc():
    import concourse.bass as bass  # noqa: F401
    import concourse.mybir as mybir
    import concourse.tile as tile
    from concourse import bacc

    f32 = mybir.dt.float32
    fp16 = mybir.dt.float16
    bf16 = mybir.dt.bfloat16

    nc = bacc.Bacc(None, target_bir_lowering=False)
    xt_d = nc.declare_dram_parameter("xt", [4, 128, B * H * W], fp16,
                                     isOutput=False)
    wp_d = nc.declare_dram_parameter("wp", [KS * KS, 128, 4 * 512], fp16,
                                     isOutput=False)
    # Tap-major output layout: out[kk, pos, 32*64].  Each inner-loop DMA
    # writes one fully contiguous block; the host transposes kk back into n.
    o_d = nc.declare_dram_parameter("out", [KS * KS, POS, 32 * FPC], bf16,
                                    isOutput=True)

    with tile.TileContext(nc) as tc:
        with (
            tc.tile_pool(name="xtp", bufs=1) as xtp,
            tc.tile_pool(name="wpp", bufs=1) as wpp,
            tc.tile_pool(name="shiftp", bufs=2) as shiftp,
            tc.tile_pool(name="stage", bufs=6) as stagep,
            tc.tile_pool(name="psum", bufs=2, space="PSUM") as psump,
        ):
            # ---- weight packs: first-consumed tap first, on the ACT ring
            # (the sync ring carries xt + all output DMAs) ----
            kk_order = [ki * 3 + kj for kj in range(3) for ki in range(3)]
            wps = [None] * (KS * KS)
            for kk in kk_order:
                wp_t = wpp.tile([128, 4 * 512], fp16, tag=f"wp{kk}",
                                name=f"wp{kk}")
                nc.scalar.dma_start(wp_t[:], wp_d[kk])
                wps[kk] = wp_t

            # ---- x: already transposed+fp16 on host; 4 per-octet tiles ----
            xts = [
                xtp.tile([128, B * H * W], fp16, tag=f"xt{o}", name=f"xt{o}")
                for o in range(4)
            ]
            for o in range(4):
                nc.sync.dma_start(xts[o][:], xt_d[o])
            xtvs = [
                t[:].rearrange("p (b h w) -> p b h w", b=B, h=H) for t in xts
            ]

            # ---- kj-shift builds: shift[(dc,a), (oct,b,h,j)] ----
            # j:14-of-16 compaction only; the three ki taps of this kj read
            # contiguous row windows.  Per-(b,oct) GPSIMD copies so early
            # matmuls start as soon as their slice lands.
            def build_shift(kj):
                sh = shiftp.tile([128, 4 * B * H * OW], fp16, tag="shift",
                                 name=f"shift{kj}")
                shv = sh[:].rearrange("p (o b h j) -> p o b h j", o=4, b=B,
                                      h=H)
                for b in range(B):
                    for o in range(4):
                        nc.gpsimd.tensor_copy(
                            shv[:, o, b], xtvs[o][:, b, :, kj:kj + OW]
                        )
                return sh

            # ---- main loop: kj outer so taps ki=0..2 reuse one shift ----
            sh_cur = build_shift(0)
            for kj in range(3):
                sh_next = build_shift(kj + 1) if kj < 2 else None
                for ki in range(3):
                    kk = ki * 3 + kj
                    for b in range(B):
                        for i0, ni in ((0, 8), (8, 6)):
                            m = ni * OW  # 112 or 84 output positions
                            ps = psump.tile([128, 2048], f32, tag="mm")
                            for o in range(4):
                                off = ((o * B + b) * H + ki + i0) * OW
                                nc.tensor.matmul(
                                    ps[0:m, o * 512:(o + 1) * 512],
                                    sh_cur[:, off: off + m],
                                    wps[kk][:, o * 512:(o + 1) * 512],
                                    start=True,
                                    stop=True,
                                )
                            st = stagep.tile([128, 2048], bf16, tag="st")
                            # f32->bf16 drain split across DVE || ACT
                            # (breakpoint balances the two engine clocks).
                            nc.vector.tensor_copy(st[0:m, 0:1024],
                                                  ps[0:m, 0:1024])
                            nc.scalar.copy(st[0:m, 1024:2048],
                                           ps[0:m, 1024:2048])
                            q0 = b * (OH * OW) + i0 * OW
                            nc.sync.dma_start(
                                o_d[kk, q0: q0 + m, :],
                                st[0:m, :],
                            )
                sh_cur = sh_next

    nc.compile()
    return nc


def _get_nc():
    if "v3" not in _NC_CACHE:
        _NC_CACHE["v3"] = _build_nc()
    return _NC_CACHE["v3"]


def make_in_maps(x, matrix):
    """Host-side input marshalling for all 8 cores."""
    x = np.ascontiguousarray(x, dtype=np.float32)
    matrix = np.ascontiguousarray(matrix, dtype=np.float32)
    # xt[oct, (dc,a), (b,h,w)] fp16
    xt = np.ascontiguousarray(
        x.transpose(3, 4, 0, 1, 2).reshape(4, 128, B * H * W)
    ).astype(np.float16)
    in_maps = []
    for c in range(NCORES):
        mr = matrix[:, :, c * FPC:(c + 1) * FPC]          # [288, 16, 64]
        mr = mr.reshape(KS * KS, 4, 8, A, FPC)            # [kk,oct,gc,a,f]
        wp = np.zeros((KS * KS, 128, 4 * 512), np.float16)
        wpv = wp.reshape(KS * KS, 8, A, 4, 8, FPC)        # [kk,gc,a,oct,gc2,f]
        for g in range(8):
            wpv[:, g, :, :, g, :] = mr[:, :, g].transpose(0, 2, 1, 3)
        in_maps.append({"xt": xt, "wp": wp})
    return in_maps


def kernel(x, matrix):
    from concourse.bass_utils import run_bass_kernel_spmd

    nc = _get_nc()
    in_maps = make_in_maps(x, matrix)
    r = run_bass_kernel_spmd(nc, in_maps, list(range(NCORES)))
    # parts[c]: [9, 784, 2048] tap-major -> [pos, kk, 32, core, 64] -> full
    arr = np.stack(
        [np.asarray(r.results[c]["out"]).astype(np.float32) for c in range(NCORES)]
    )
    arr = arr.reshape(NCORES, KS * KS, POS, 32, FPC)
    arr = arr.transpose(2, 1, 3, 0, 4)               # [pos, kk, 32, core, f]
    full = arr.reshape(POS, NCAP, FTOT)
    return np.ascontiguousarray(
        full.reshape(B, OH, OW, NCAP, 32, 16).astype(np.float32)
    )
